# revision 28
# baseline (speedup 1.0000x reference)
"""DMTet marching-tetrahedra kernel for 8 Trainium2 NeuronCores.

Split of work:
  - Kernel A (device, 8 cores, tet-sharded): deformed positions
    pos = verts_grid + tanh(deform)/128 (vertex-sharded), per-tet edge
    extraction: tetindex, per-edge sorted endpoints (a,b), edge
    valid*crossing masks.
  - Host: exact sort/unique of the 9M edge keys (rank/inverse
    computation) and permutation application (posf[ua], posf[ub]).
  - Kernel B (device, 8 cores, slot- and tet-sharded): sign-crossing
    interpolation to produce verts[9M,3]; triangle-table assembly of
    faces[1.5M,2,3].
"""

import numpy as np

import concourse.bass as bass
import concourse.tile as tile
from concourse import bacc, mybir
from concourse.bass_utils import run_bass_kernel_spmd

F32 = mybir.dt.float32
I32 = mybir.dt.int32
OP = mybir.AluOpType

# Problem constants (hardcoded per harness contract)
GRID_RES = 128
N_VERTS = 300000
N_TETS = 1500000
K = 6 * N_TETS
SENT = N_VERTS * N_VERTS
NCORES = 8

# Padded / sharded sizes
T_SH = N_TETS // NCORES            # 187500 tets per core
T_PAD = 187520                     # = 128 * 1465
TCOLS = T_PAD // 128               # 1465
V_PAD = 300032                     # = 128 * 2344
V_SH = V_PAD // NCORES             # 37504 verts per core
VCOLS = V_SH // 128                # 293
S_SH = K // NCORES                 # 1125000 slots per core
S_PAD = 1125120                    # = 128 * 8790
SCOLS = S_PAD // 128               # 8790
SC_CAP = 524288                    # compacted crossing slots per core = 128*4096
SC_COLS = SC_CAP // 128            # 4096
SC_TOT = SC_CAP * NCORES           # 4194304 global capacity (~3.94M expected)
TV_CAP = 172032                    # compacted valid tets per core = 128*1344
TV_COLS = TV_CAP // 128            # 1344
TV_TOT = TV_CAP * NCORES           # 1376256 (~1.3125M expected)
T2_CAP = 73728                     # compacted ntri==2 tets per core = 128*576
T2_COLS = T2_CAP // 128            # 576
T2_TOT = T2_CAP * NCORES           # 589824 (~562.5K expected)

TRIANGLE_TABLE = np.array([
    [-1, -1, -1, -1, -1, -1], [1, 0, 2, -1, -1, -1], [4, 0, 3, -1, -1, -1],
    [1, 4, 2, 1, 3, 4], [3, 1, 5, -1, -1, -1], [2, 3, 0, 2, 5, 3],
    [1, 4, 0, 1, 5, 4], [4, 2, 5, -1, -1, -1], [4, 5, 2, -1, -1, -1],
    [4, 1, 0, 4, 5, 1], [3, 2, 0, 3, 5, 2], [1, 3, 5, -1, -1, -1],
    [4, 1, 2, 4, 3, 1], [3, 0, 4, -1, -1, -1], [2, 0, 1, -1, -1, -1],
    [-1, -1, -1, -1, -1, -1]], dtype=np.int32)
NUM_TRIANGLES = np.array([0, 1, 1, 2, 1, 2, 2, 1, 1, 2, 2, 1, 2, 1, 1, 0],
                         dtype=np.int32)
EDGE_I = [0, 0, 0, 1, 1, 2]
EDGE_J = [1, 2, 3, 2, 3, 3]

_CACHE = {}
TRACE = False
LAST_EXEC_NS = {}


def _view(dram, p=128):
    """Flat [M] or [M, C] DRAM tensor viewed as [128, M/128(, C)]."""
    ap = dram[:]
    if len(ap.shape) == 1:
        return ap.rearrange("(p m) -> p m", p=p)
    return ap.rearrange("(p m) c -> p m c", p=p)


def build_kernel_a():
    nc = bacc.Bacc("TRN2", target_bir_lowering=False, debug=False)
    vtx = nc.dram_tensor("vtx", [4, T_PAD], I32, kind="ExternalInput")
    occ4 = nc.dram_tensor("occ4", [4, T_PAD], F32, kind="ExternalInput")
    vg = nc.dram_tensor("vg", [3, V_SH], F32, kind="ExternalInput")
    df = nc.dram_tensor("df", [3, V_SH], F32, kind="ExternalInput")
    sdfs = nc.dram_tensor("sdfs", [V_SH], F32, kind="ExternalInput")

    ab = nc.dram_tensor("ab", [12, T_PAD], I32, kind="ExternalOutput")
    ti_out = nc.dram_tensor("ti", [T_PAD], F32, kind="ExternalOutput")
    posf = nc.dram_tensor("posf", [V_SH, 4], F32, kind="ExternalOutput")

    with tile.TileContext(nc) as tc:
        with tc.tile_pool(name="vpool", bufs=1) as vp, \
             tc.tile_pool(name="tpool", bufs=4) as tp:
            # --- pos/sdf table (vertex shard), all columns at once (293) ---
            pf = vp.tile([128, VCOLS, 4], F32)
            for c in range(3):
                vg_t = vp.tile([128, VCOLS], F32, tag="vg")
                df_t = vp.tile([128, VCOLS], F32, tag="df")
                nc.sync.dma_start(vg_t[:], _view(vg[c]))
                nc.sync.dma_start(df_t[:], _view(df[c]))
                th = vp.tile([128, VCOLS], F32, tag="th")
                nc.scalar.activation(th[:], df_t[:],
                                     mybir.ActivationFunctionType.Tanh)
                # pos_c = vg + tanh(df)/GRID_RES
                nc.vector.scalar_tensor_tensor(
                    pf[:, :, c], th[:], 1.0 / GRID_RES, vg_t[:],
                    op0=OP.mult, op1=OP.add)
            sd_t = vp.tile([128, VCOLS], F32, tag="sd")
            nc.sync.dma_start(sd_t[:], _view(sdfs))
            nc.vector.tensor_copy(pf[:, :, 3], sd_t[:])
            nc.sync.dma_start(_view(posf), pf[:])

            # --- per-tet edge extraction, chunked over columns ---
            W = 512
            for c0 in range(0, TCOLS, W):
                w = min(W, TCOLS - c0)
                cs = slice(c0, c0 + w)
                v_t = []
                o_t = []
                for j in range(4):
                    vj = tp.tile([128, w], I32, tag=f"v{j}")
                    nc.sync.dma_start(vj[:], _view(vtx[j])[:, cs])
                    v_t.append(vj)
                    oj = tp.tile([128, w], F32, tag=f"o{j}")
                    nc.scalar.dma_start(oj[:], _view(occ4[j])[:, cs])
                    o_t.append(oj)
                # tetindex = o0 + 2 o1 + 4 o2 + 8 o3
                t01 = tp.tile([128, w], F32, tag="t01")
                nc.vector.scalar_tensor_tensor(
                    t01[:], o_t[1][:], 2.0, o_t[0][:], op0=OP.mult, op1=OP.add)
                t23 = tp.tile([128, w], F32, tag="t23")
                nc.vector.tensor_scalar_mul(t23[:], o_t[2][:], 4.0)
                nc.vector.scalar_tensor_tensor(
                    t23[:], o_t[3][:], 8.0, t23[:], op0=OP.mult, op1=OP.add)
                ti_t = tp.tile([128, w], F32, tag="ti")
                nc.vector.tensor_add(ti_t[:], t01[:], t23[:])
                nc.scalar.dma_start(_view(ti_out)[:, cs], ti_t[:])
                for e in range(6):
                    i, j = EDGE_I[e], EDGE_J[e]
                    amin = tp.tile([128, w], I32, tag=f"amin{e % 2}")
                    bmax = tp.tile([128, w], I32, tag=f"bmax{e % 2}")
                    nc.vector.tensor_tensor(amin[:], v_t[i][:], v_t[j][:], op=OP.min)
                    nc.vector.tensor_tensor(bmax[:], v_t[i][:], v_t[j][:], op=OP.max)
                    nc.sync.dma_start(_view(ab[e])[:, cs], amin[:])
                    nc.scalar.dma_start(_view(ab[6 + e])[:, cs], bmax[:])
    nc.compile()
    return nc


def build_kernel_b():
    nc = bacc.Bacc("TRN2", target_bir_lowering=False, debug=False)
    pap = nc.dram_tensor("pap", [4, SC_CAP], F32, kind="ExternalInput")
    pbp = nc.dram_tensor("pbp", [4, SC_CAP], F32, kind="ExternalInput")
    gv = nc.dram_tensor("gv", [6, TV_CAP], F32, kind="ExternalInput")
    trv = nc.dram_tensor("trv", [3, TV_CAP], F32, kind="ExternalInput")
    gw = nc.dram_tensor("gw", [6, T2_CAP], F32, kind="ExternalInput")
    trw = nc.dram_tensor("trw", [3, T2_CAP], F32, kind="ExternalInput")

    vout = nc.dram_tensor("vout", [3, SC_CAP], F32, kind="ExternalOutput")
    facesv = nc.dram_tensor("facesv", [3, TV_CAP], F32, kind="ExternalOutput")
    facesw = nc.dram_tensor("facesw", [3, T2_CAP], F32, kind="ExternalOutput")

    ACT_COPY = mybir.ActivationFunctionType.Copy

    def faces_stream(tc, fp, g6_d, tr_d, out_d, ncols, wchunk):
            for c0 in range(0, ncols, wchunk):
                w = min(wchunk, ncols - c0)
                cs = slice(c0, c0 + w)
                g_t = []
                for e in range(6):
                    g = fp.tile([128, w], F32, tag=f"g{e}", name=f"g{e}")
                    eng = nc.sync if e % 2 == 0 else nc.scalar
                    eng.dma_start(g[:], _view(g6_d[e])[:, cs])
                    g_t.append(g)
                for c in range(3):
                    tr = fp.tile([128, w], F32, tag=f"tr{c % 2}")
                    nc.sync.dma_start(tr[:], _view(tr_d[c])[:, cs])
                    acc = fp.tile([128, w], F32, tag=f"acc{c % 2}")
                    tmp2 = fp.tile([128, w], F32, tag=f"tmp{c % 2}")
                    for e in range(6):
                        dst = acc if e == 0 else tmp2
                        nc.vector.scalar_tensor_tensor(
                            dst[:], tr[:], float(e), g_t[e][:],
                            op0=OP.is_equal, op1=OP.mult)
                        if e > 0:
                            nc.vector.tensor_add(acc[:], acc[:], tmp2[:])
                    facc = fp.tile([128, w], F32, tag=f"facc{c % 2}")
                    nc.scalar.activation(facc[:], acc[:], ACT_COPY, bias=-1.0)
                    nc.scalar.dma_start(_view(out_d[c])[:, cs], facc[:])

    with tile.TileContext(nc) as tc:
        # All pools open together so the scheduler can interleave the
        # faces chains with the interpolation stream (fills DVE gaps).
        with tc.tile_pool(name="fp", bufs=2) as fp, \
             tc.tile_pool(name="spool", bufs=2) as sp:
            # --- faces: c<3 over valid tets, c>=3 over ntri==2 tets ---
            faces_stream(tc, fp, gv, trv, facesv, TV_COLS, 672)
            faces_stream(tc, fp, gw, trw, facesw, T2_COLS, 576)

            # --- interpolation over compacted crossing slots (no masking:
            # every slot crosses, so denom = sa - sb directly) ---
            W = 512  # SC_COLS = 4096 = 8 * 512
            for c0 in range(0, SC_COLS, W):
                w = min(W, SC_COLS - c0)
                cs = slice(c0, c0 + w)
                pa_t = [sp.tile([128, w], F32, tag=f"pa{c}", name=f"pa{c}")
                        for c in range(4)]
                pb_t = [sp.tile([128, w], F32, tag=f"pb{c}", name=f"pb{c}")
                        for c in range(4)]
                for c in range(4):
                    nc.sync.dma_start(pa_t[c][:], _view(pap[c])[:, cs])
                    nc.scalar.dma_start(pb_t[c][:], _view(pbp[c])[:, cs])
                sa = pa_t[3]
                sb = pb_t[3]
                d0 = sp.tile([128, w], F32, tag="d0")
                nc.vector.tensor_tensor(d0[:], sa[:], sb[:], op=OP.subtract)
                rcp = sp.tile([128, w], F32, tag="rcp")
                nc.vector.reciprocal_approx_fast(rcp[:], d0[:])
                wa = sp.tile([128, w], F32, tag="wa")
                nc.vector.scalar_tensor_tensor(wa[:], sb[:], -1.0, rcp[:],
                                               op0=OP.mult, op1=OP.mult)
                wb = sp.tile([128, w], F32, tag="wb")
                nc.vector.tensor_tensor(wb[:], sa[:], rcp[:], op=OP.mult)
                tmp = sp.tile([128, w], F32, tag="tmp")
                for c in range(3):
                    vo = sp.tile([128, w], F32, tag=f"vo{c}", name=f"vo{c}")
                    nc.vector.tensor_tensor(tmp[:], pa_t[c][:], wa[:], op=OP.mult)
                    nc.vector.tensor_tensor(vo[:], pb_t[c][:], wb[:], op=OP.mult)
                    nc.vector.tensor_add(vo[:], vo[:], tmp[:])
                    nc.sync.dma_start(_view(vout[c])[:, cs], vo[:])
    nc.compile()
    return nc


def _get_kernels():
    if "a" not in _CACHE:
        _CACHE["a"] = build_kernel_a()
    if "b" not in _CACHE:
        _CACHE["b"] = build_kernel_b()
    return _CACHE["a"], _CACHE["b"]


def _pad1(x, n):
    out = np.zeros((n,) + x.shape[1:], x.dtype)
    out[: x.shape[0]] = x
    return out


def kernel(verts_grid, sdf, deform, tet):
    verts_grid = np.asarray(verts_grid, np.float32)
    sdf = np.asarray(sdf, np.float32)
    deform = np.asarray(deform, np.float32)
    tet = np.asarray(tet)
    nc_a, nc_b = _get_kernels()

    tet32 = tet.astype(np.int32)
    occ = sdf > 0.0
    occ4 = occ[tet32].astype(np.float32)          # [T, 4]

    vg_p = _pad1(verts_grid, V_PAD)               # [V_PAD, 3]
    df_p = _pad1(deform, V_PAD)
    sdf_p = _pad1(sdf, V_PAD)

    in_maps = []
    for k in range(NCORES):
        ts = slice(k * T_SH, (k + 1) * T_SH)
        vs = slice(k * V_SH, (k + 1) * V_SH)
        in_maps.append({
            "vtx": _pad1(tet32[ts], T_PAD).T.copy(),       # [4, T_PAD]
            "occ4": _pad1(occ4[ts], T_PAD).T.copy(),       # [4, T_PAD]
            "vg": vg_p[vs].T.copy(),                       # [3, V_SH]
            "df": df_p[vs].T.copy(),
            "sdfs": sdf_p[vs].copy(),
        })
    res_a = run_bass_kernel_spmd(nc_a, in_maps, core_ids=list(range(NCORES)),
                                 trace=TRACE)
    if TRACE:
        LAST_EXEC_NS["a"] = res_a.exec_time_ns

    ab = np.concatenate([r["ab"][:, :T_SH] for r in res_a.results], axis=1)
    ti = np.concatenate([r["ti"][:T_SH] for r in res_a.results])
    posf = np.concatenate([r["posf"] for r in res_a.results])   # [V_PAD, 4]

    tia = np.rint(ti).astype(np.int32)
    valid = (tia != 0) & (tia != 15)                           # [T]
    a6 = ab[:6].astype(np.int64)                               # [6, T]
    b6 = ab[6:].astype(np.int64)
    keys = a6 * N_VERTS + b6                                   # [6, T]
    keys = np.where(valid[None, :], keys, SENT)
    keys_flat = keys.T.reshape(-1)                             # [K] in (t, e) order

    uq, inv = np.unique(keys_flat, return_inverse=True)
    u = len(uq)
    ua = np.zeros(K, np.int64)
    ub = np.zeros(K, np.int64)
    real = uq != SENT
    ua[:u][real] = uq[real] // N_VERTS
    ub[:u][real] = uq[real] % N_VERTS

    # crossing slots only (ua=ub=0 for SENT slots -> never crossing)
    cross = occ[ua] != occ[ub]                                 # [K]
    cross_idx = np.flatnonzero(cross)
    ncross = len(cross_idx)
    n_dev = min(ncross, SC_TOT)
    pap_all = np.zeros((4, SC_TOT), np.float32)
    pbp_all = np.zeros((4, SC_TOT), np.float32)
    pap_all[:, :n_dev] = posf[ua[cross_idx[:n_dev]]].T
    pbp_all[:, :n_dev] = posf[ub[cross_idx[:n_dev]]].T
    pap_all[3, n_dev:] = 1.0                                   # pad: denom = 2
    pbp_all[3, n_dev:] = -1.0

    # faces inputs: maskv from tetindex bits, g = maskv * (inv + 1)
    ei = np.array(EDGE_I)
    ej = np.array(EDGE_J)
    bits = ((tia[:, None] >> np.arange(4)[None, :]) & 1).astype(np.int8)
    mv6 = (bits[:, ei] != bits[:, ej]) & valid[:, None]        # [T, 6]
    inv6 = inv.reshape(-1, 6)
    g6 = (mv6 * (inv6 + 1)).astype(np.float32)                 # [T, 6]

    tri = TRIANGLE_TABLE[tia]                                  # [T, 6]
    ntri = np.where(valid, NUM_TRIANGLES[tia], 0)              # [T]
    cidx = np.arange(6) // 3
    sel = (cidx[None, :] < ntri[:, None]) & (tri >= 0)
    trism = np.where(sel, tri, -1).astype(np.float32)          # [T, 6]

    # compact: c<3 only matters for valid tets, c>=3 only for ntri==2 tets
    valid_idx = np.flatnonzero(valid)
    n2_idx = np.flatnonzero(ntri == 2)
    nv_dev = min(len(valid_idx), TV_TOT)
    n2_dev = min(len(n2_idx), T2_TOT)
    gv_all = np.zeros((6, TV_TOT), np.float32)
    gv_all[:, :nv_dev] = g6[valid_idx[:nv_dev]].T
    trv_all = np.full((3, TV_TOT), -1.0, np.float32)
    trv_all[:, :nv_dev] = trism[valid_idx[:nv_dev], :3].T
    gw_all = np.zeros((6, T2_TOT), np.float32)
    gw_all[:, :n2_dev] = g6[n2_idx[:n2_dev]].T
    trw_all = np.full((3, T2_TOT), -1.0, np.float32)
    trw_all[:, :n2_dev] = trism[n2_idx[:n2_dev], 3:].T

    in_maps_b = []
    for k in range(NCORES):
        cs = slice(k * SC_CAP, (k + 1) * SC_CAP)
        vs_ = slice(k * TV_CAP, (k + 1) * TV_CAP)
        ws_ = slice(k * T2_CAP, (k + 1) * T2_CAP)
        in_maps_b.append({
            "pap": np.ascontiguousarray(pap_all[:, cs]),
            "pbp": np.ascontiguousarray(pbp_all[:, cs]),
            "gv": np.ascontiguousarray(gv_all[:, vs_]),
            "trv": np.ascontiguousarray(trv_all[:, vs_]),
            "gw": np.ascontiguousarray(gw_all[:, ws_]),
            "trw": np.ascontiguousarray(trw_all[:, ws_]),
        })
    res_b = run_bass_kernel_spmd(nc_b, in_maps_b, core_ids=list(range(NCORES)),
                                 trace=TRACE)
    if TRACE:
        LAST_EXEC_NS["b"] = res_b.exec_time_ns

    vout = np.concatenate([r["vout"] for r in res_b.results], axis=1)  # [3, SC_TOT]
    verts = np.zeros((K, 3), np.float32)
    verts[cross_idx[:n_dev]] = vout[:, :n_dev].T
    if ncross > SC_TOT:
        # capacity overflow fallback (statistically impossible for this
        # problem's input distribution): finish the tail on host
        rest = cross_idx[SC_TOT:]
        sa = sdf[ua[rest]]
        sb = sdf[ub[rest]]
        den = sa - sb
        wa = -sb / den
        wb = sa / den
        pos = posf[:, :3]
        verts[rest] = pos[ua[rest]] * wa[:, None] + pos[ub[rest]] * wb[:, None]
    fv = np.concatenate([r["facesv"] for r in res_b.results], axis=1)  # [3, TV_TOT]
    fw = np.concatenate([r["facesw"] for r in res_b.results], axis=1)  # [3, T2_TOT]
    faces6 = np.full((N_TETS, 6), -1, np.int64)
    faces6[valid_idx[:nv_dev], :3] = fv[:, :nv_dev].T
    faces6[n2_idx[:n2_dev], 3:] = fw[:, :n2_dev].T
    if len(valid_idx) > TV_TOT or len(n2_idx) > T2_TOT:
        # capacity overflow fallback (statistically impossible): host math
        for rows, cols in ((valid_idx[TV_TOT:], slice(0, 3)),
                           (n2_idx[T2_TOT:], slice(3, 6))):
            if len(rows):
                trs = trism[rows][:, cols].astype(np.int64)
                gat = np.take_along_axis(g6[rows].astype(np.int64),
                                         np.clip(trs, 0, 5), axis=1)
                faces6[rows[:, None], np.arange(6)[cols][None, :]] = np.where(
                    trs >= 0, gat - 1, -1)
    faces = faces6.reshape(-1, 2, 3)
    return verts.astype(np.float32), faces


# revision 29
# speedup vs baseline: 1.0533x; 1.0533x over previous
"""DMTet marching-tetrahedra kernel for 8 Trainium2 NeuronCores.

Split of work:
  - Kernel A (device, 8 cores, tet-sharded): deformed positions
    pos = verts_grid + tanh(deform)/128 (vertex-sharded), per-tet edge
    extraction: tetindex, per-edge sorted endpoints (a,b), edge
    valid*crossing masks.
  - Host: exact sort/unique of the 9M edge keys (rank/inverse
    computation) and permutation application (posf[ua], posf[ub]).
  - Kernel B (device, 8 cores, slot- and tet-sharded): sign-crossing
    interpolation to produce verts[9M,3]; triangle-table assembly of
    faces[1.5M,2,3].
"""

import numpy as np

import concourse.bass as bass
import concourse.tile as tile
from concourse import bacc, mybir
from concourse.bass_utils import run_bass_kernel_spmd

F32 = mybir.dt.float32
I32 = mybir.dt.int32
OP = mybir.AluOpType

# Problem constants (hardcoded per harness contract)
GRID_RES = 128
N_VERTS = 300000
N_TETS = 1500000
K = 6 * N_TETS
SENT = N_VERTS * N_VERTS
NCORES = 8

# Padded / sharded sizes
T_SH = N_TETS // NCORES            # 187500 tets per core
T_PAD = 187520                     # = 128 * 1465
TCOLS = T_PAD // 128               # 1465
V_PAD = 300032                     # = 128 * 2344
V_SH = V_PAD // NCORES             # 37504 verts per core
VCOLS = V_SH // 128                # 293
S_SH = K // NCORES                 # 1125000 slots per core
S_PAD = 1125120                    # = 128 * 8790
SCOLS = S_PAD // 128               # 8790
SC_CAP = 524288                    # compacted crossing slots per core = 128*4096
SC_COLS = SC_CAP // 128            # 4096
SC_TOT = SC_CAP * NCORES           # 4194304 global capacity (~3.94M expected)
TV_CAP = 172032                    # compacted valid tets per core = 128*1344
TV_COLS = TV_CAP // 128            # 1344
TV_TOT = TV_CAP * NCORES           # 1376256 (~1.3125M expected)
T2_CAP = 73728                     # compacted ntri==2 tets per core = 128*576
T2_COLS = T2_CAP // 128            # 576
T2_TOT = T2_CAP * NCORES           # 589824 (~562.5K expected)

TRIANGLE_TABLE = np.array([
    [-1, -1, -1, -1, -1, -1], [1, 0, 2, -1, -1, -1], [4, 0, 3, -1, -1, -1],
    [1, 4, 2, 1, 3, 4], [3, 1, 5, -1, -1, -1], [2, 3, 0, 2, 5, 3],
    [1, 4, 0, 1, 5, 4], [4, 2, 5, -1, -1, -1], [4, 5, 2, -1, -1, -1],
    [4, 1, 0, 4, 5, 1], [3, 2, 0, 3, 5, 2], [1, 3, 5, -1, -1, -1],
    [4, 1, 2, 4, 3, 1], [3, 0, 4, -1, -1, -1], [2, 0, 1, -1, -1, -1],
    [-1, -1, -1, -1, -1, -1]], dtype=np.int32)
NUM_TRIANGLES = np.array([0, 1, 1, 2, 1, 2, 2, 1, 1, 2, 2, 1, 2, 1, 1, 0],
                         dtype=np.int32)
EDGE_I = [0, 0, 0, 1, 1, 2]
EDGE_J = [1, 2, 3, 2, 3, 3]

_CACHE = {}
TRACE = False
LAST_EXEC_NS = {}


def _view(dram, p=128):
    """Flat [M] or [M, C] DRAM tensor viewed as [128, M/128(, C)]."""
    ap = dram[:]
    if len(ap.shape) == 1:
        return ap.rearrange("(p m) -> p m", p=p)
    return ap.rearrange("(p m) c -> p m c", p=p)


def build_kernel_a():
    nc = bacc.Bacc("TRN2", target_bir_lowering=False, debug=False)
    vtx = nc.dram_tensor("vtx", [4, T_PAD], I32, kind="ExternalInput")
    occ4 = nc.dram_tensor("occ4", [4, T_PAD], F32, kind="ExternalInput")
    vg = nc.dram_tensor("vg", [3, V_SH], F32, kind="ExternalInput")
    df = nc.dram_tensor("df", [3, V_SH], F32, kind="ExternalInput")
    sdfs = nc.dram_tensor("sdfs", [V_SH], F32, kind="ExternalInput")

    ab = nc.dram_tensor("ab", [12, T_PAD], I32, kind="ExternalOutput")
    ti_out = nc.dram_tensor("ti", [T_PAD], F32, kind="ExternalOutput")
    posf = nc.dram_tensor("posf", [V_SH, 4], F32, kind="ExternalOutput")

    with tile.TileContext(nc) as tc:
        with tc.tile_pool(name="vpool", bufs=1) as vp, \
             tc.tile_pool(name="tpool", bufs=4) as tp:
            # --- pos/sdf table (vertex shard), all columns at once (293) ---
            pf = vp.tile([128, VCOLS, 4], F32)
            for c in range(3):
                vg_t = vp.tile([128, VCOLS], F32, tag="vg")
                df_t = vp.tile([128, VCOLS], F32, tag="df")
                nc.sync.dma_start(vg_t[:], _view(vg[c]))
                nc.sync.dma_start(df_t[:], _view(df[c]))
                th = vp.tile([128, VCOLS], F32, tag="th")
                nc.scalar.activation(th[:], df_t[:],
                                     mybir.ActivationFunctionType.Tanh)
                # pos_c = vg + tanh(df)/GRID_RES
                nc.vector.scalar_tensor_tensor(
                    pf[:, :, c], th[:], 1.0 / GRID_RES, vg_t[:],
                    op0=OP.mult, op1=OP.add)
            sd_t = vp.tile([128, VCOLS], F32, tag="sd")
            nc.sync.dma_start(sd_t[:], _view(sdfs))
            nc.vector.tensor_copy(pf[:, :, 3], sd_t[:])
            nc.sync.dma_start(_view(posf), pf[:])

            # --- per-tet edge extraction, chunked over columns ---
            W = 733
            for c0 in range(0, TCOLS, W):
                w = min(W, TCOLS - c0)
                cs = slice(c0, c0 + w)
                v_t = []
                o_t = []
                for j in range(4):
                    vj = tp.tile([128, w], I32, tag=f"v{j}")
                    nc.sync.dma_start(vj[:], _view(vtx[j])[:, cs])
                    v_t.append(vj)
                    oj = tp.tile([128, w], F32, tag=f"o{j}")
                    nc.scalar.dma_start(oj[:], _view(occ4[j])[:, cs])
                    o_t.append(oj)
                # tetindex = o0 + 2 o1 + 4 o2 + 8 o3
                t01 = tp.tile([128, w], F32, tag="t01")
                nc.vector.scalar_tensor_tensor(
                    t01[:], o_t[1][:], 2.0, o_t[0][:], op0=OP.mult, op1=OP.add)
                t23 = tp.tile([128, w], F32, tag="t23")
                nc.vector.tensor_scalar_mul(t23[:], o_t[2][:], 4.0)
                nc.vector.scalar_tensor_tensor(
                    t23[:], o_t[3][:], 8.0, t23[:], op0=OP.mult, op1=OP.add)
                ti_t = tp.tile([128, w], F32, tag="ti")
                nc.vector.tensor_add(ti_t[:], t01[:], t23[:])
                nc.scalar.dma_start(_view(ti_out)[:, cs], ti_t[:])
                for e in range(6):
                    i, j = EDGE_I[e], EDGE_J[e]
                    amin = tp.tile([128, w], I32, tag=f"amin{e % 2}")
                    bmax = tp.tile([128, w], I32, tag=f"bmax{e % 2}")
                    nc.vector.tensor_tensor(amin[:], v_t[i][:], v_t[j][:], op=OP.min)
                    nc.vector.tensor_tensor(bmax[:], v_t[i][:], v_t[j][:], op=OP.max)
                    nc.sync.dma_start(_view(ab[e])[:, cs], amin[:])
                    nc.scalar.dma_start(_view(ab[6 + e])[:, cs], bmax[:])
    nc.compile()
    return nc


def build_kernel_b():
    nc = bacc.Bacc("TRN2", target_bir_lowering=False, debug=False)
    pap = nc.dram_tensor("pap", [4, SC_CAP], F32, kind="ExternalInput")
    pbp = nc.dram_tensor("pbp", [4, SC_CAP], F32, kind="ExternalInput")
    gv = nc.dram_tensor("gv", [6, TV_CAP], F32, kind="ExternalInput")
    trv = nc.dram_tensor("trv", [3, TV_CAP], F32, kind="ExternalInput")
    gw = nc.dram_tensor("gw", [6, T2_CAP], F32, kind="ExternalInput")
    trw = nc.dram_tensor("trw", [3, T2_CAP], F32, kind="ExternalInput")

    vout = nc.dram_tensor("vout", [3, SC_CAP], F32, kind="ExternalOutput")
    facesv = nc.dram_tensor("facesv", [3, TV_CAP], F32, kind="ExternalOutput")
    facesw = nc.dram_tensor("facesw", [3, T2_CAP], F32, kind="ExternalOutput")

    ACT_COPY = mybir.ActivationFunctionType.Copy

    def faces_stream(tc, fp, g6_d, tr_d, out_d, ncols, wchunk):
            for c0 in range(0, ncols, wchunk):
                w = min(wchunk, ncols - c0)
                cs = slice(c0, c0 + w)
                g_t = []
                for e in range(6):
                    g = fp.tile([128, w], F32, tag=f"g{e}", name=f"g{e}")
                    eng = nc.sync if e % 2 == 0 else nc.scalar
                    eng.dma_start(g[:], _view(g6_d[e])[:, cs])
                    g_t.append(g)
                for c in range(3):
                    tr = fp.tile([128, w], F32, tag=f"tr{c % 2}")
                    nc.sync.dma_start(tr[:], _view(tr_d[c])[:, cs])
                    acc = fp.tile([128, w], F32, tag=f"acc{c % 2}")
                    tmp2 = fp.tile([128, w], F32, tag=f"tmp{c % 2}")
                    for e in range(6):
                        dst = acc if e == 0 else tmp2
                        nc.vector.scalar_tensor_tensor(
                            dst[:], tr[:], float(e), g_t[e][:],
                            op0=OP.is_equal, op1=OP.mult)
                        if e > 0:
                            nc.vector.tensor_add(acc[:], acc[:], tmp2[:])
                    facc = fp.tile([128, w], F32, tag=f"facc{c % 2}")
                    nc.scalar.activation(facc[:], acc[:], ACT_COPY, bias=-1.0)
                    nc.scalar.dma_start(_view(out_d[c])[:, cs], facc[:])

    with tile.TileContext(nc) as tc:
        # All pools open together so the scheduler can interleave the
        # faces chains with the interpolation stream (fills DVE gaps).
        with tc.tile_pool(name="fp", bufs=2) as fp, \
             tc.tile_pool(name="spool", bufs=2) as sp:
            # --- faces: c<3 over valid tets, c>=3 over ntri==2 tets ---
            faces_stream(tc, fp, gv, trv, facesv, TV_COLS, 672)
            faces_stream(tc, fp, gw, trw, facesw, T2_COLS, 576)

            # --- interpolation over compacted crossing slots (no masking:
            # every slot crosses, so denom = sa - sb directly) ---
            W = 512  # SC_COLS = 4096 = 8 * 512
            for c0 in range(0, SC_COLS, W):
                w = min(W, SC_COLS - c0)
                cs = slice(c0, c0 + w)
                pa_t = [sp.tile([128, w], F32, tag=f"pa{c}", name=f"pa{c}")
                        for c in range(4)]
                pb_t = [sp.tile([128, w], F32, tag=f"pb{c}", name=f"pb{c}")
                        for c in range(4)]
                for c in range(4):
                    nc.sync.dma_start(pa_t[c][:], _view(pap[c])[:, cs])
                    nc.scalar.dma_start(pb_t[c][:], _view(pbp[c])[:, cs])
                sa = pa_t[3]
                sb = pb_t[3]
                d0 = sp.tile([128, w], F32, tag="d0")
                nc.vector.tensor_tensor(d0[:], sa[:], sb[:], op=OP.subtract)
                rcp = sp.tile([128, w], F32, tag="rcp")
                nc.vector.reciprocal_approx_fast(rcp[:], d0[:])
                wa = sp.tile([128, w], F32, tag="wa")
                nc.vector.scalar_tensor_tensor(wa[:], sb[:], -1.0, rcp[:],
                                               op0=OP.mult, op1=OP.mult)
                wb = sp.tile([128, w], F32, tag="wb")
                nc.vector.tensor_tensor(wb[:], sa[:], rcp[:], op=OP.mult)
                tmp = sp.tile([128, w], F32, tag="tmp")
                for c in range(3):
                    vo = sp.tile([128, w], F32, tag=f"vo{c}", name=f"vo{c}")
                    nc.vector.tensor_tensor(tmp[:], pa_t[c][:], wa[:], op=OP.mult)
                    nc.vector.tensor_tensor(vo[:], pb_t[c][:], wb[:], op=OP.mult)
                    nc.vector.tensor_add(vo[:], vo[:], tmp[:])
                    nc.sync.dma_start(_view(vout[c])[:, cs], vo[:])
    nc.compile()
    return nc


def _get_kernels():
    if "a" not in _CACHE:
        _CACHE["a"] = build_kernel_a()
    if "b" not in _CACHE:
        _CACHE["b"] = build_kernel_b()
    return _CACHE["a"], _CACHE["b"]


def _pad1(x, n):
    out = np.zeros((n,) + x.shape[1:], x.dtype)
    out[: x.shape[0]] = x
    return out


def kernel(verts_grid, sdf, deform, tet):
    verts_grid = np.asarray(verts_grid, np.float32)
    sdf = np.asarray(sdf, np.float32)
    deform = np.asarray(deform, np.float32)
    tet = np.asarray(tet)
    nc_a, nc_b = _get_kernels()

    tet32 = tet.astype(np.int32)
    occ = sdf > 0.0
    occ4 = occ[tet32].astype(np.float32)          # [T, 4]

    vg_p = _pad1(verts_grid, V_PAD)               # [V_PAD, 3]
    df_p = _pad1(deform, V_PAD)
    sdf_p = _pad1(sdf, V_PAD)

    in_maps = []
    for k in range(NCORES):
        ts = slice(k * T_SH, (k + 1) * T_SH)
        vs = slice(k * V_SH, (k + 1) * V_SH)
        in_maps.append({
            "vtx": _pad1(tet32[ts], T_PAD).T.copy(),       # [4, T_PAD]
            "occ4": _pad1(occ4[ts], T_PAD).T.copy(),       # [4, T_PAD]
            "vg": vg_p[vs].T.copy(),                       # [3, V_SH]
            "df": df_p[vs].T.copy(),
            "sdfs": sdf_p[vs].copy(),
        })
    res_a = run_bass_kernel_spmd(nc_a, in_maps, core_ids=list(range(NCORES)),
                                 trace=TRACE)
    if TRACE:
        LAST_EXEC_NS["a"] = res_a.exec_time_ns

    ab = np.concatenate([r["ab"][:, :T_SH] for r in res_a.results], axis=1)
    ti = np.concatenate([r["ti"][:T_SH] for r in res_a.results])
    posf = np.concatenate([r["posf"] for r in res_a.results])   # [V_PAD, 4]

    tia = np.rint(ti).astype(np.int32)
    valid = (tia != 0) & (tia != 15)                           # [T]
    a6 = ab[:6].astype(np.int64)                               # [6, T]
    b6 = ab[6:].astype(np.int64)
    keys = a6 * N_VERTS + b6                                   # [6, T]
    keys = np.where(valid[None, :], keys, SENT)
    keys_flat = keys.T.reshape(-1)                             # [K] in (t, e) order

    uq, inv = np.unique(keys_flat, return_inverse=True)
    u = len(uq)
    ua = np.zeros(K, np.int64)
    ub = np.zeros(K, np.int64)
    real = uq != SENT
    ua[:u][real] = uq[real] // N_VERTS
    ub[:u][real] = uq[real] % N_VERTS

    # crossing slots only (ua=ub=0 for SENT slots -> never crossing)
    cross = occ[ua] != occ[ub]                                 # [K]
    cross_idx = np.flatnonzero(cross)
    ncross = len(cross_idx)
    n_dev = min(ncross, SC_TOT)
    pap_all = np.zeros((4, SC_TOT), np.float32)
    pbp_all = np.zeros((4, SC_TOT), np.float32)
    pap_all[:, :n_dev] = posf[ua[cross_idx[:n_dev]]].T
    pbp_all[:, :n_dev] = posf[ub[cross_idx[:n_dev]]].T
    pap_all[3, n_dev:] = 1.0                                   # pad: denom = 2
    pbp_all[3, n_dev:] = -1.0

    # faces inputs: maskv from tetindex bits, g = maskv * (inv + 1)
    ei = np.array(EDGE_I)
    ej = np.array(EDGE_J)
    bits = ((tia[:, None] >> np.arange(4)[None, :]) & 1).astype(np.int8)
    mv6 = (bits[:, ei] != bits[:, ej]) & valid[:, None]        # [T, 6]
    inv6 = inv.reshape(-1, 6)
    g6 = (mv6 * (inv6 + 1)).astype(np.float32)                 # [T, 6]

    tri = TRIANGLE_TABLE[tia]                                  # [T, 6]
    ntri = np.where(valid, NUM_TRIANGLES[tia], 0)              # [T]
    cidx = np.arange(6) // 3
    sel = (cidx[None, :] < ntri[:, None]) & (tri >= 0)
    trism = np.where(sel, tri, -1).astype(np.float32)          # [T, 6]

    # compact: c<3 only matters for valid tets, c>=3 only for ntri==2 tets
    valid_idx = np.flatnonzero(valid)
    n2_idx = np.flatnonzero(ntri == 2)
    nv_dev = min(len(valid_idx), TV_TOT)
    n2_dev = min(len(n2_idx), T2_TOT)
    gv_all = np.zeros((6, TV_TOT), np.float32)
    gv_all[:, :nv_dev] = g6[valid_idx[:nv_dev]].T
    trv_all = np.full((3, TV_TOT), -1.0, np.float32)
    trv_all[:, :nv_dev] = trism[valid_idx[:nv_dev], :3].T
    gw_all = np.zeros((6, T2_TOT), np.float32)
    gw_all[:, :n2_dev] = g6[n2_idx[:n2_dev]].T
    trw_all = np.full((3, T2_TOT), -1.0, np.float32)
    trw_all[:, :n2_dev] = trism[n2_idx[:n2_dev], 3:].T

    in_maps_b = []
    for k in range(NCORES):
        cs = slice(k * SC_CAP, (k + 1) * SC_CAP)
        vs_ = slice(k * TV_CAP, (k + 1) * TV_CAP)
        ws_ = slice(k * T2_CAP, (k + 1) * T2_CAP)
        in_maps_b.append({
            "pap": np.ascontiguousarray(pap_all[:, cs]),
            "pbp": np.ascontiguousarray(pbp_all[:, cs]),
            "gv": np.ascontiguousarray(gv_all[:, vs_]),
            "trv": np.ascontiguousarray(trv_all[:, vs_]),
            "gw": np.ascontiguousarray(gw_all[:, ws_]),
            "trw": np.ascontiguousarray(trw_all[:, ws_]),
        })
    res_b = run_bass_kernel_spmd(nc_b, in_maps_b, core_ids=list(range(NCORES)),
                                 trace=TRACE)
    if TRACE:
        LAST_EXEC_NS["b"] = res_b.exec_time_ns

    vout = np.concatenate([r["vout"] for r in res_b.results], axis=1)  # [3, SC_TOT]
    verts = np.zeros((K, 3), np.float32)
    verts[cross_idx[:n_dev]] = vout[:, :n_dev].T
    if ncross > SC_TOT:
        # capacity overflow fallback (statistically impossible for this
        # problem's input distribution): finish the tail on host
        rest = cross_idx[SC_TOT:]
        sa = sdf[ua[rest]]
        sb = sdf[ub[rest]]
        den = sa - sb
        wa = -sb / den
        wb = sa / den
        pos = posf[:, :3]
        verts[rest] = pos[ua[rest]] * wa[:, None] + pos[ub[rest]] * wb[:, None]
    fv = np.concatenate([r["facesv"] for r in res_b.results], axis=1)  # [3, TV_TOT]
    fw = np.concatenate([r["facesw"] for r in res_b.results], axis=1)  # [3, T2_TOT]
    faces6 = np.full((N_TETS, 6), -1, np.int64)
    faces6[valid_idx[:nv_dev], :3] = fv[:, :nv_dev].T
    faces6[n2_idx[:n2_dev], 3:] = fw[:, :n2_dev].T
    if len(valid_idx) > TV_TOT or len(n2_idx) > T2_TOT:
        # capacity overflow fallback (statistically impossible): host math
        for rows, cols in ((valid_idx[TV_TOT:], slice(0, 3)),
                           (n2_idx[T2_TOT:], slice(3, 6))):
            if len(rows):
                trs = trism[rows][:, cols].astype(np.int64)
                gat = np.take_along_axis(g6[rows].astype(np.int64),
                                         np.clip(trs, 0, 5), axis=1)
                faces6[rows[:, None], np.arange(6)[cols][None, :]] = np.where(
                    trs >= 0, gat - 1, -1)
    faces = faces6.reshape(-1, 2, 3)
    return verts.astype(np.float32), faces
